# revision 31
# baseline (speedup 1.0000x reference)
"""Trainium2 Bass kernel for nn_CPADConvOffsetStage.

The reference module is:
  up_posi = grid_sample_bilinear_border(posi_map -> [B,16,GP,GP], grid = base + offset*scale)
  h       = relu(w1 @ up_posi + b1)           (1x1 conv)
  weights = (w2 @ h + b2).reshape(B,64,9,H,W) (1x1 conv -> per-pixel 3x3 kernels)
  x_adapt = w_ca @ x                          (1x1 conv)
  out     = sum_k weights[:,:,k] * unfold3x3(x_adapt)[:,:,k] + bias

In setup_inputs() posi_map is spatially constant per channel (jnp.ones).
Bilinear interpolation weights sum to exactly 1, so up_posi is spatially
constant => h, weights are spatially constant => the whole module reduces
to ONE dense 3x3 convolution with host-precomputable weights
    Wfull[o,c,k] = wk[o,k] * w_ca[o,c],   wk = (w2 @ relu(w1 @ v + b1) + b2)
plus the bias.  The kernel below runs that conv data-parallel over batch
(1 batch image per NeuronCore, 8 cores).

v2 schedule (from ntff profile of v1 @51us):
  * whole padded image resident in SBUF (one [128, 130, 130] bf16 tile,
    top half = x, bottom half = x shifted left one column) loaded by 4
    big row-slab DMAs (per-partition-contiguous, ~9KB chunks) -- v1 spent
    ~34us in packetized 4.7KB-chunk DMAs and stalled the PE mid-kernel.
  * weights DMA issued FIRST (v1 queued it behind two x blocks: first
    matmul only at t=15.3us).
  * dummy warm-up matmuls + a dummy activation during the DMA fill keep
    the PE HAM un-throttled (v1 ran most matmuls at the 1.2GHz cold
    clock: 416ns instead of 216ns each) and pull the 1.3us
    ACT_TABLE_LOAD off the critical path.
  * no A+B psum fold: each PE column group accumulates a FULL tile of
    its own (block pair 2b / 2b+1), so eviction is a single scalar-engine
    activation per [128,512] psum bank (bias fused) -- v1's per-tile DVE
    tensor_tensor fold was a ~22us co-bottleneck.
  * bf16 output (harness tolerance 2e-2; bf16 rounding adds ~4e-3),
    halving output DMA bytes; outputs leave as 8 DMAs with 4KB/partition
    contiguous chunks.

If posi_map is NOT per-channel spatially constant (never the case for the
shipped setup_inputs), we fall back to an exact numpy port of the
reference.
"""

import os
import numpy as np
from contextlib import ExitStack

import concourse.bass as bass
import concourse.tile as tile
from concourse import mybir
from concourse.bass_utils import run_bass_kernel_spmd

# Problem constants (hardcoded per contract)
B, C, H, W = 8, 64, 128, 128
OC = 64
KK = 3
POSI_CH, GP = 16, 16
NCORES = 8
F32 = mybir.dt.float32
BF16 = mybir.dt.bfloat16

HPAD, WPAD = H + 2, W + 2      # host-padded image (130 x 130)
ROWS_PER_TILE = 4              # 4 rows * 128 cols = 512 = one fp32 psum bank
NWARM = 11                     # HAM warm-up matmuls issued during DMA fill

_cached_nc = None
_cached_variant = None
last_results = None            # test harness introspection


def _ensure_ntff_hook():
    """Register the axon NTFF-profile hook that this image's antenv lacks."""
    import sys
    import types

    if "antenv.axon_hooks" in sys.modules:
        return
    try:
        from trn_agent_boot.trn_boot import _ntff_profile_via_ctypes

        hook = _ntff_profile_via_ctypes("/opt/axon/libaxon_pjrt.so")
    except Exception:
        hook = None
    mod = types.ModuleType("antenv.axon_hooks")
    mod.get_axon_ntff_profile_hook = lambda: hook
    mod.set_axon_ntff_profile_hook = lambda h: None
    sys.modules["antenv.axon_hooks"] = mod
    try:
        import antenv

        antenv.axon_hooks = mod
    except Exception:
        pass


def _build_conv_nc_v2(split_waits=True):
    """3x3 conv, 64->64 ch, on one host-padded [64,130,130] bf16 image.

    SPMD over 8 cores, one batch image per core.  Tap k -> dup3 packing:
    partitions 64:128 of the input tile hold the image shifted one column
    left, so a K=128 matmul contracts two horizontally-adjacent taps at
    once; the three column-2 solo taps run K=128 with zero bottom-half
    weights.  Column group h0 accumulates all 6 matmuls for block 2b,
    h64 for block 2b+1 concurrently (both full 3x3 results -- no fold).
    """
    nc = bass.Bass()
    x_d = nc.declare_dram_parameter("x", [C, HPAD, WPAD], BF16, isOutput=False)
    w_d = nc.declare_dram_parameter("wts", [128, 5 * OC], BF16, isOutput=False)
    b_d = nc.declare_dram_parameter("wb", [128, 1], F32, isOutput=False)
    o_d = nc.declare_dram_parameter("out", [OC, H, W], BF16, isOutput=True)

    # input row-slabs: small first slab so block 0 (needs rows 0:18) can
    # start computing as early as possible
    SLABS = [(0, 18), (18, 34), (34, 50), (50, 66), (66, 87), (87, 108), (108, 130)]

    with ExitStack() as ctx:
        tc = ctx.enter_context(tile.TileContext(nc))
        singles = ctx.enter_context(tc.tile_pool(name="singles", bufs=1))
        outs = ctx.enter_context(tc.tile_pool(name="outs", bufs=2))
        psum = ctx.enter_context(tc.tile_pool(name="psum", bufs=4, space="PSUM"))
        wpsum = ctx.enter_context(tc.tile_pool(name="wpsum", bufs=1, space="PSUM"))

        # --- whole-image input tiles:
        #   xb : top = x, bottom = x shifted left one COLUMN
        #   xb2: top = x, bottom = x shifted up one ROW
        # column-adjacent tap pairs contract via xb; the column-2 taps
        # (0,2)+(1,2) are row-adjacent and contract via xb2.  All matmuls
        # stay K=128 / 128x64 tiling mode (row-tiled K-splits into one
        # psum bank are illegal -- concurrent row tiles must not touch
        # the same bank).
        xb = singles.tile([128, HPAD, WPAD], BF16, name="xb")
        xb2 = singles.tile([128, HPAD, WPAD], BF16, name="xb2")
        # dup's last column is never written by the shift copy; zero it so
        # the K=128 solo matmul (zero bottom-half weights) can't hit NaN*0
        nc.vector.memset(xb[C:128, :, WPAD - 1 : WPAD], 0.0)
        w_sb = singles.tile([128, 5 * OC], BF16)
        b_sb = singles.tile([128, 1], F32)
        # first slab first (smallest latency to first compute), weights
        # next, then the rest of the image
        nc.sync.dma_start(out=xb[0:C, 0:18, :], in_=x_d[:, 0:18, :])
        nc.sync.dma_start(out=w_sb[:, :], in_=w_d[:, :])
        nc.sync.dma_start(out=b_sb[:, :], in_=b_d[:, :])
        # all input slabs on the sync ring, in demand order -- sharing the
        # SDMA engines with a second ring delays the latency-critical
        # first slabs (measured: slab0 +2us, stream start +1.9us)
        for r0, r1 in SLABS[1:]:
            nc.sync.dma_start(out=xb[0:C, r0:r1, :], in_=x_d[:, r0:r1, :])
        for si, (r0, r1) in enumerate(SLABS):
            if si == 0:
                # split the first dup copy so the very first matmul
                # (needs rows 0:7 only) is gated by a half-size copy
                nc.vector.tensor_copy(
                    xb[C:128, 0:9, 0 : WPAD - 1], xb[0:C, 0:9, 1:WPAD]
                )
                nc.vector.tensor_copy(
                    xb[C:128, 9:18, 0 : WPAD - 1], xb[0:C, 9:18, 1:WPAD]
                )
            else:
                nc.vector.tensor_copy(
                    xb[C:128, r0:r1, 0 : WPAD - 1],
                    xb[0:C, r0:r1, 1:WPAD],
                )
            nc.vector.tensor_copy(xb2[0:C, r0:r1, :], xb[0:C, r0:r1, :])
            # xb2 bottom rows r hold x row r+1; shift the copy window down
            # one row so it reads ONLY this slab (no cross-slab dependency)
            ra = max(r0 - 1, 0)
            nc.vector.tensor_copy(
                xb2[C:128, ra : r1 - 1, :], xb[0:C, ra + 1 : r1, :]
            )

        # --- warm-up fodder (zeros; gated only on memsets, not DMA)
        warm_w = singles.tile([128, OC], BF16, name="warm_w")
        nc.gpsimd.memset(warm_w[:, :], 0.0)
        warm_x = singles.tile([128, 512], BF16, name="warm_x")
        nc.gpsimd.memset(warm_x[:, :], 0.0)
        warm_o = singles.tile([128, 1], BF16, name="warm_o")

        # --- HAM warm-up: keep the PE busy (and load the ACT table) while
        # the input DMAs land, so real matmuls run at the full clock
        nc.scalar.activation(
            out=warm_o[:, :],
            in_=warm_x[:, 0:1],
            func=mybir.ActivationFunctionType.Identity,
            bias=0.0,
            scale=1.0,
        )
        ps_warm = wpsum.tile([128, 512], F32)
        for i in range(NWARM):
            nc.tensor.matmul(
                ps_warm[0:OC, :],
                lhsT=warm_w[:, :],
                rhs=warm_x[:, :],
                start=True,
                stop=True,
                tile_position=(0, 0),
                skip_group_check=True,
            )

        # Matmul schedule per psum tile (output rows r0..r0+4), all K=128:
        #   wi 0..2: pairs (3p, 3p+1) via xb's column-shifted duplicate
        #   wi 3   : pair (0,2)+(1,2) via xb2's row-shifted duplicate
        #   wi 4   : solo (2,2) via xb, zero bottom-half weights
        # 5 matmul slots per tile pair instead of 6.
        for b in range(8):  # 16-row blocks
            out_blk = outs.tile([128, 2, 512], BF16)
            for j in range(2):  # psum pairs: h0 = rows 8j.., h64 = rows 8+8j..
                ps = psum.tile([128, 512], F32)
                r0s = [16 * b + 4 * j, 16 * b + 8 + 4 * j]
                mms = [
                    (0, xb, 0, 0),   # taps (0,0)+(0,1)
                    (1, xb, 1, 0),   # taps (1,0)+(1,1)
                    (2, xb, 2, 0),   # taps (2,0)+(2,1)
                    (3, xb2, 0, 2),  # taps (0,2)+(1,2)
                    (4, xb, 2, 2),   # tap  (2,2), bottom weights zero
                ]
                for wi, src, ri, ci in mms:
                    for h in range(2):
                        r0 = r0s[h]
                        nc.tensor.matmul(
                            ps[64 * h : 64 * h + OC, :],
                            lhsT=w_sb[:, wi * OC : (wi + 1) * OC],
                            rhs=src[:, r0 + ri : r0 + ri + ROWS_PER_TILE, ci : ci + W],
                            start=(wi == 0),
                            stop=(wi == 4),
                            tile_position=(0, 64 * h),
                            skip_group_check=True,
                        )
                if b == 7 and j == 1:
                    # final eviction on DVE (idle by now) so it runs in
                    # parallel with ACT's pair-j0 eviction -- shortens the
                    # last-matmul -> last-DMA tail
                    nc.vector.tensor_scalar_add(
                        out_blk[:, j, :], ps[:, :], b_sb[:, 0:1]
                    )
                else:
                    nc.scalar.activation(
                        out=out_blk[:, j, :],
                        in_=ps[:, :],
                        func=mybir.ActivationFunctionType.Identity,
                        bias=b_sb[:, 0:1],
                        scale=1.0,
                    )
            # 8-row output DMAs; h64 half goes out on the scalar HWDGE
            # ring so the final pair drains in parallel.  The first two
            # blocks' outputs fire while input slabs are still streaming
            # on the sync ring -- route them to the scalar ring so they
            # don't delay the late slabs.
            h0_eng = nc.scalar if b < 2 else nc.sync
            h0_eng.dma_start(
                out=o_d[:, 16 * b : 16 * b + 8, :],
                in_=out_blk[0:OC].rearrange(
                    "p j (i w) -> p (j i) w", i=ROWS_PER_TILE
                ),
            )
            nc.scalar.dma_start(
                out=o_d[:, 16 * b + 8 : 16 * b + 16, :],
                in_=out_blk[OC:128].rearrange(
                    "p j (i w) -> p (j i) w", i=ROWS_PER_TILE
                ),
            )
    if split_waits:
        _split_sync_waits(nc)
    return nc


def _split_sync_waits(nc, limit=1):
    """Hoist extra sync waits onto injected wait-only EventSemaphore ops.

    The neuronxcc walrus used under axon rejects compute instructions
    carrying more than one sync wait ("Too many sync wait commands").
    For every instruction with >limit waits, keep the first `limit` and
    prepend one wait-only EventSemaphore per extra wait on the same
    engine (same program position => same semantics).
    """
    import copy as _copy

    f = nc.m.functions[0]
    template = None
    for blk in f.blocks:
        for inst in blk.instructions:
            if type(inst).__name__ == "InstEventSemaphore":
                template = inst
                break
        if template is not None:
            break
    if template is None:
        return
    n_split = 0
    for blk in f.blocks:
        new_list = []
        changed = False
        for inst in blk.instructions:
            si = getattr(inst, "sync_info", None)
            waits = list(si.on_wait) if (si and si.on_wait) else []
            if len(waits) > limit:
                for w in waits[limit:]:
                    ev = _copy.deepcopy(template)
                    ev.name = f"waitsplit_{n_split}"
                    n_split += 1
                    ev.engine = inst.engine
                    ev.sync_info = mybir.SyncInfo(on_wait=[w], on_update=[])
                    new_list.append(ev)
                inst.sync_info = mybir.SyncInfo(
                    on_wait=waits[:limit], on_update=list(si.on_update or [])
                )
                changed = True
            new_list.append(inst)
        if changed:
            blk.instructions = new_list


def _host_conv_weights(posi_map, w1, b1, w2, b2, w_ca, bias):
    """Collapse the constant-posi_map weight generator on the host."""
    pm = np.asarray(posi_map, np.float64)[0]              # [16, GP, GP]
    vvec = pm.reshape(POSI_CH, -1)[:, 0]                  # per-channel constant
    h = np.maximum(np.asarray(w1, np.float64) @ vvec + np.asarray(b1, np.float64), 0.0)
    wvec = np.asarray(w2, np.float64) @ h + np.asarray(b2, np.float64)   # [576]
    wk = wvec.reshape(OC, 9)                              # [o, k]
    wca = np.asarray(w_ca, np.float64)                    # [o, c]
    wfull = wk[:, None, :] * wca[:, :, None]              # [o, c, k]
    wts = np.ascontiguousarray(
        wfull.transpose(1, 2, 0).reshape(C, 9 * OC).astype(np.float32)
    )                                                     # [c, k*OC + o]
    wb = np.ascontiguousarray(
        np.asarray(bias, np.float32).reshape(OC, 1)
    )
    return wts, wb


def _pack_v3(wts):
    """Repack [C, 9*OC] tap-major lhsT into the v3 layout [128, 5*OC].

    Columns 0:3*OC are K=128 pairs (taps (3p, 3p+1) stacked on the
    partition axis, matching the +1-column-shifted input duplicate).
    Column group 3: tap (0,2) on the top partitions, tap (1,2) on the
    bottom (used by two concurrent K=64 row-tiled matmuls).  Column
    group 4: tap (2,2) on top, zero bottom.
    """
    w3 = np.zeros((128, 5 * OC), np.float32)
    for p in range(3):
        w3[0:C, p * OC:(p + 1) * OC] = wts[:, (3 * p) * OC:(3 * p + 1) * OC]
        w3[C:2 * C, p * OC:(p + 1) * OC] = wts[:, (3 * p + 1) * OC:(3 * p + 2) * OC]
    w3[0:C, 3 * OC:4 * OC] = wts[:, 2 * OC:3 * OC]          # tap (0,2)
    w3[C:2 * C, 3 * OC:4 * OC] = wts[:, 5 * OC:6 * OC]      # tap (1,2)
    w3[0:C, 4 * OC:5 * OC] = wts[:, 8 * OC:9 * OC]          # tap (2,2)
    return w3


def _numpy_reference(x, offset, posi_map, w1, b1, w2, b2, w_ca, bias):
    """Exact numpy port of reference.py (general-input fallback)."""
    x = np.asarray(x, np.float32)
    offset = np.asarray(offset, np.float32)
    posi_map = np.asarray(posi_map, np.float32)
    w1 = np.asarray(w1, np.float32)
    b1 = np.asarray(b1, np.float32)
    w2 = np.asarray(w2, np.float32)
    b2 = np.asarray(b2, np.float32)
    w_ca = np.asarray(w_ca, np.float32)
    bias = np.asarray(bias, np.float32)

    Bq, _, Hq, Wq = x.shape
    dx = offset[:, 0] * (2.0 / max(Wq - 1, 1)) * 0.5
    dy = offset[:, 1] * (2.0 / max(Hq - 1, 1)) * 0.5
    ys = np.linspace(-1.0, 1.0, Hq, dtype=x.dtype)
    xs = np.linspace(-1.0, 1.0, Wq, dtype=x.dtype)
    gx = xs[None, None, :] + dx
    gy = ys[None, :, None] + dy
    img = np.broadcast_to(posi_map, (Bq, posi_map.shape[1], GP, GP))

    Hp = Wp = GP
    imgT = img.transpose(0, 2, 3, 1)                      # [B, Hp, Wp, C]
    ix = np.clip((gx + 1.0) * 0.5 * (Wp - 1), 0.0, Wp - 1)
    iy = np.clip((gy + 1.0) * 0.5 * (Hp - 1), 0.0, Hp - 1)
    x0 = np.floor(ix).astype(np.int32)
    y0 = np.floor(iy).astype(np.int32)
    x1 = np.minimum(x0 + 1, Wp - 1)
    y1 = np.minimum(y0 + 1, Hp - 1)
    wx = (ix - x0.astype(ix.dtype))[..., None]
    wy = (iy - y0.astype(iy.dtype))[..., None]
    bb = np.arange(Bq)[:, None, None]
    v00 = imgT[bb, y0, x0]
    v01 = imgT[bb, y0, x1]
    v10 = imgT[bb, y1, x0]
    v11 = imgT[bb, y1, x1]
    top = v00 * (1 - wx) + v01 * wx
    bot = v10 * (1 - wx) + v11 * wx
    up = (top * (1 - wy) + bot * wy).transpose(0, 3, 1, 2)  # [B, 16, H, W]

    h = np.maximum(np.einsum('oc,bchw->bohw', w1, up) + b1[None, :, None, None], 0.0)
    weights = np.einsum('oc,bchw->bohw', w2, h) + b2[None, :, None, None]
    weights = weights.reshape(Bq, OC, KK * KK, Hq, Wq)
    x_adapt = np.einsum('oc,bchw->bohw', w_ca, x)
    xp = np.pad(x_adapt, ((0, 0), (0, 0), (1, 1), (1, 1)))
    patches = np.stack(
        [xp[:, :, i:i + Hq, j:j + Wq] for i in range(KK) for j in range(KK)],
        axis=2,
    )
    out = (weights * patches).sum(axis=2) + bias
    return out.astype(np.float32)


def kernel(**inputs):
    global _cached_nc, _cached_variant, last_results
    x = np.ascontiguousarray(np.asarray(inputs["x"], np.float32))
    posi_map = np.asarray(inputs["posi_map"], np.float32)

    per_ch = posi_map.reshape(posi_map.shape[0] * posi_map.shape[1], -1)
    if not np.all(per_ch == per_ch[:, :1]):
        # general (spatially varying posi_map) fallback: exact numpy port
        return _numpy_reference(**{k: inputs[k] for k in (
            "x", "offset", "posi_map", "w1", "b1", "w2", "b2", "w_ca", "bias")})

    wts, wb = _host_conv_weights(
        posi_map, inputs["w1"], inputs["b1"], inputs["w2"], inputs["b2"],
        inputs["w_ca"], inputs["bias"],
    )

    if _cached_nc is None:
        _cached_nc = _build_conv_nc_v2()
        _cached_variant = "v2"

    import ml_dtypes

    w3 = _pack_v3(wts).astype(ml_dtypes.bfloat16)
    wb2 = np.ascontiguousarray(np.concatenate([wb, wb], axis=0))  # [128, 1]
    xpad = np.pad(x, ((0, 0), (0, 0), (1, 1), (1, 1))).astype(ml_dtypes.bfloat16)

    in_maps = [{"x": xpad[i], "wts": w3, "wb": wb2} for i in range(NCORES)]
    trace = os.environ.get("BASS_KERNEL_TRACE", "0") == "1"
    if trace:
        _ensure_ntff_hook()
    res = run_bass_kernel_spmd(
        _cached_nc, in_maps, list(range(NCORES)), trace=trace
    )
    last_results = res
    out = np.stack(
        [np.asarray(res.results[i]["out"]).astype(np.float32) for i in range(NCORES)],
        axis=0,
    )
    return out
